# revision 1
# baseline (speedup 1.0000x reference)
"""2-layer GAT (GATConv x2, PyG-style) on 8 Trainium2 NeuronCores.

Strategy (edge-parallel, dst-sharded):
  - Nodes padded to NP = 8*98*64 = 50176 and sharded contiguously: core c
    owns nodes [c*6272, (c+1)*6272), i.e. 98 windows of W=64 dst nodes.
  - Edges (incl. self loops) are sorted by dst window on the host; each core
    processes exactly the edges that land in its dst windows, so no
    cross-core reduction of messages is needed.
  - Node phase: each core computes rows [h | s_src | s_dst] = x @ Wcat for
    its node slice, then an AllGather builds the full gather table in DRAM.
  - Edge phase: per 64-dst-node window, edges are processed in blocks of
    128 (one edge per partition).  Indirect DMA gathers [h|s_src] rows by
    src id and s_dst by dst id.  Scores e = leakyrelu(sS+sD), p = exp(e)
    (no segment-max needed: scores are bounded, exp stays in f32 range).
    A one-hot selection matrix (is_equal vs iota) + PE matmul accumulates
    both the denominator sum(p) and the messages sum(p * h_src) into PSUM
    per dst slot; the softmax division happens once per dst row at drain.
  - Per-core window->slot assignment is sorted by edge count so all cores
    share one SPMD program (slot block counts = max over cores of the
    order statistics).  The resulting per-core node permutation is folded
    into the layer-2 gather indices; the host un-permutes the output.
"""

import numpy as np

P = 128          # edges per block / SBUF partitions
W = 64           # dst nodes per window
NC = 8           # cores
WPC = 98         # windows per core
NPC = WPC * W    # nodes per core (6272)
NP = NC * NPC    # padded node count (50176)
IN_DIM = 128
HEADS1, HID1 = 8, 8
OUT_DIM = 64
NEG_SLOPE = 0.2
SUPER_BLK = 72   # max gather blocks per indirect-DMA super instruction


def _mk_head_mat(a):
    """[H, C] attention vector -> [H*C, H] block-diagonal matrix."""
    H, C = a.shape
    A = np.zeros((H * C, H), np.float32)
    for h in range(H):
        A[h * C:(h + 1) * C, h] = a[h]
    return A


def _prep(x, edge_index, W1, a_src1, a_dst1, b1, W2, a_src2, a_dst2, b2,
          n_cores=NC, wpc=WPC):
    """Host-side preprocessing. Returns (cfg, in_maps, perm)."""
    npc = wpc * W
    n_pad = n_cores * npc
    n = x.shape[0]
    assert n <= n_pad

    x = np.asarray(x, np.float32)
    xp = np.zeros((n_pad, IN_DIM), np.float32)
    xp[:n] = x

    ei = np.asarray(edge_index)
    src = np.concatenate([ei[0], np.arange(n)]).astype(np.int64)
    dst = np.concatenate([ei[1], np.arange(n)]).astype(np.int64)

    # sort edges by destination window
    win = (dst // W).astype(np.int64)
    order = np.argsort(win, kind="stable")
    src, dst, win = src[order], dst[order], win[order]
    nw = n_pad // W
    counts = np.bincount(win, minlength=nw)
    starts = np.concatenate([[0], np.cumsum(counts)])

    counts_c = counts.reshape(n_cores, wpc)
    K_c = np.ceil(counts_c / P).astype(np.int64)          # blocks per window
    orders = [np.argsort(-counts_c[c], kind="stable") for c in range(n_cores)]
    Ks = np.max(np.stack([K_c[c][orders[c]] for c in range(n_cores)]), axis=0)
    Ks = np.maximum(Ks, 1)  # keep every slot non-degenerate
    Mtot = int(Ks.sum())

    # perm[node] = global row in the slot-ordered (layer-2) table
    perm = np.empty(n_pad, np.int64)
    for c in range(n_cores):
        inv = np.empty(wpc, np.int64)
        inv[orders[c]] = np.arange(wpc)
        wl = np.arange(wpc)
        base = (c * wpc + wl) * W
        for woff in range(W):
            perm[base + woff] = c * npc + inv * W + woff

    def pack(arrs, dtype):
        # per-slot flat arrays -> [128, Mtot] with edge j*128+p at [p, j]
        cols = [a.reshape(-1, P).T for a in arrs]
        return np.ascontiguousarray(np.concatenate(cols, axis=1), dtype)

    in_maps = []
    for c in range(n_cores):
        esrc, edstg, esrc2, edst2, edstl = [], [], [], [], []
        for s in range(wpc):
            wloc = orders[c][s]
            wglob = c * wpc + wloc
            e0, e1 = starts[wglob], starts[wglob + 1]
            nslots = int(Ks[s]) * P
            npad = nslots - (e1 - e0)
            sw = src[e0:e1]
            dw = dst[e0:e1]
            z = np.zeros(npad, np.int64)
            esrc.append(np.concatenate([sw, z]))
            edstg.append(np.concatenate([dw, z]))
            esrc2.append(np.concatenate([perm[sw], z]))
            edst2.append(np.concatenate([perm[dw], z]))
            edstl.append(np.concatenate([dw - wglob * W,
                                         np.full(npad, W, np.int64)]))
        in_maps.append({
            "x": np.ascontiguousarray(xp[c * npc:(c + 1) * npc]),
            "esrc": pack(esrc, np.int32),
            "edst": pack(edstg, np.int32),
            "esrc2": pack(esrc2, np.int32),
            "edst2": pack(edst2, np.int32),
            "edstl": pack(edstl, np.float32),
        })

    W1 = np.asarray(W1, np.float32)
    W2 = np.asarray(W2, np.float32)
    wc1 = np.concatenate([W1, W1 @ _mk_head_mat(np.asarray(a_src1, np.float32)),
                          W1 @ _mk_head_mat(np.asarray(a_dst1, np.float32))],
                         axis=1)                     # [128, 80]
    wc2 = np.concatenate([W2, W2 @ np.asarray(a_src2, np.float32).T,
                          W2 @ np.asarray(a_dst2, np.float32).T], axis=1)  # [64, 66]
    b1r = np.tile(np.asarray(b1, np.float32)[None, :], (W, 1))
    b2r = np.tile(np.asarray(b2, np.float32)[None, :], (W, 1))
    for m in in_maps:
        m["wc1"] = np.ascontiguousarray(wc1, np.float32)
        m["wc2"] = np.ascontiguousarray(wc2, np.float32)
        m["b1r"] = np.ascontiguousarray(b1r, np.float32)
        m["b2r"] = np.ascontiguousarray(b2r, np.float32)

    cfg = dict(n_cores=n_cores, wpc=wpc, npc=npc, n_pad=n_pad,
               Ks=[int(k) for k in Ks], Mtot=Mtot)
    return cfg, in_maps, perm


def _sub(apbase, off, dims):
    """Custom multi-level free-dim AP on top of a tile's [:, :] AP."""
    import concourse.bass as bass
    return bass.AP(tensor=apbase.tensor, offset=apbase.offset + off,
                   ap=[list(apbase.ap[0])] + [list(d) for d in dims])


def _build(nc, cfg, debug_tabs=False, reps=1):
    """Emit the full SPMD program into nc. Returns nothing."""
    import concourse.bass as bass
    import concourse.mybir as mybir
    import concourse.tile as tile
    from concourse.bass import IndirectOffsetOnAxis

    f32 = mybir.dt.float32
    i32 = mybir.dt.int32
    Alu = mybir.AluOpType
    Act = mybir.ActivationFunctionType

    n_cores, wpc, npc, n_pad = cfg["n_cores"], cfg["wpc"], cfg["npc"], cfg["n_pad"]
    Ks, Mtot = cfg["Ks"], cfg["Mtot"]
    groups = [list(range(n_cores))]

    # --- dram I/O ---
    x_d = nc.dram_tensor("x", [npc, IN_DIM], f32, kind="ExternalInput")
    esrc_d = nc.dram_tensor("esrc", [P, Mtot], i32, kind="ExternalInput")
    edst_d = nc.dram_tensor("edst", [P, Mtot], i32, kind="ExternalInput")
    esrc2_d = nc.dram_tensor("esrc2", [P, Mtot], i32, kind="ExternalInput")
    edst2_d = nc.dram_tensor("edst2", [P, Mtot], i32, kind="ExternalInput")
    edstl_d = nc.dram_tensor("edstl", [P, Mtot], f32, kind="ExternalInput")
    wc1_d = nc.dram_tensor("wc1", [IN_DIM, 80], f32, kind="ExternalInput")
    wc2_d = nc.dram_tensor("wc2", [64, 66], f32, kind="ExternalInput")
    b1r_d = nc.dram_tensor("b1r", [W, 64], f32, kind="ExternalInput")
    b2r_d = nc.dram_tensor("b2r", [W, 64], f32, kind="ExternalInput")
    out_d = nc.dram_tensor("out", [npc, OUT_DIM], f32, kind="ExternalOutput")

    shared = "Local"
    t1s_d = nc.dram_tensor("t1slice", [npc, 80], f32, kind="Internal")
    table1 = nc.dram_tensor("table1", [n_pad, 80], f32, kind="Internal",
                            addr_space=shared)
    t2s_d = nc.dram_tensor("t2slice", [npc, 66], f32, kind="Internal")
    table2 = nc.dram_tensor("table2", [n_pad, 66], f32, kind="Internal",
                            addr_space=shared)

    if debug_tabs:
        dbg1_d = nc.dram_tensor("dbg1", [n_pad, 80], f32, kind="ExternalOutput")
        dbg2_d = nc.dram_tensor("dbg2", [n_pad, 66], f32, kind="ExternalOutput")

    ident_d = nc.inline_tensor(np.eye(P, dtype=np.float32), "ident")
    iota_d = nc.inline_tensor(
        np.tile(np.arange(W, dtype=np.float32), (P, 1)), "iotaw")

    # supers: greedy grouping of slots by block budget
    supers = []  # list of (slot_start, nslots, blk_start, nblk)
    s0, b0 = 0, 0
    s = 0
    while s < wpc:
        nb = 0
        s0 = s
        while s < wpc and nb + Ks[s] <= SUPER_BLK:
            nb += Ks[s]
            s += 1
        supers.append((s0, s - s0, b0, nb))
        b0 += nb
    assert b0 == Mtot

    nt = npc // P  # node tiles per core

    with tile.TileContext(nc) as tc:
        with tc.tile_pool(name="const", bufs=1) as cp, \
             tc.tile_pool(name="work", bufs=3) as wp, \
             tc.tile_pool(name="gath", bufs=3) as gp, \
             tc.tile_pool(name="ohp", bufs=2) as op_, \
             tc.tile_pool(name="drain", bufs=3) as dp, \
             tc.tile_pool(name="eps", bufs=4, space="PSUM") as pp, \
             tc.tile_pool(name="nps", bufs=2, space="PSUM") as np_:

            ident = cp.tile([P, P], f32, tag="ident")
            nc.sync.dma_start(out=ident[:, :], in_=ident_d[:, :])
            iota = cp.tile([P, W], f32, tag="iota")
            nc.sync.dma_start(out=iota[:, :], in_=iota_d[:, :])
            wc1 = cp.tile([IN_DIM, 80], f32, tag="wc1")
            nc.sync.dma_start(out=wc1[:, :], in_=wc1_d[:, :])
            wc2 = cp.tile([64, 66], f32, tag="wc2")
            nc.sync.dma_start(out=wc2[:, :], in_=wc2_d[:, :])
            b1r = cp.tile([W, 64], f32, tag="b1r")
            nc.sync.dma_start(out=b1r[:, :], in_=b1r_d[:, :])
            b2r = cp.tile([W, 64], f32, tag="b2r")
            nc.sync.dma_start(out=b2r[:, :], in_=b2r_d[:, :])

            esrc = cp.tile([P, Mtot], i32, tag="esrc")
            nc.sync.dma_start(out=esrc[:, :], in_=esrc_d[:, :])
            edst = cp.tile([P, Mtot], i32, tag="edst")
            nc.sync.dma_start(out=edst[:, :], in_=edst_d[:, :])
            esrc2 = cp.tile([P, Mtot], i32, tag="esrc2")
            nc.sync.dma_start(out=esrc2[:, :], in_=esrc2_d[:, :])
            edst2 = cp.tile([P, Mtot], i32, tag="edst2")
            nc.sync.dma_start(out=edst2[:, :], in_=edst2_d[:, :])
            edstl = cp.tile([P, Mtot], f32, tag="edstl")
            nc.sync.dma_start(out=edstl[:, :], in_=edstl_d[:, :])

            h2big = cp.tile([P, (wpc // 2) * W], f32, tag="h2big")
            import os as _os0
            if _os0.environ.get("K_ABLATE", ""):
                nc.gpsimd.memset(h2big[:, :], 0.0)

            for _rep in range(reps):
                _build_body(nc, cfg, locals())


def _build_body(nc, cfg, env):
    import concourse.bass as bass
    import concourse.mybir as mybir
    from concourse.bass import IndirectOffsetOnAxis

    f32 = mybir.dt.float32
    Alu = mybir.AluOpType
    Act = mybir.ActivationFunctionType
    (n_pad, wpc, npc) = (cfg["n_pad"], cfg["wpc"], cfg["npc"])
    Ks = cfg["Ks"]
    groups = env["groups"]
    supers = env["supers"]
    nt = env["nt"]
    debug_tabs = env["debug_tabs"]
    x_d, t1s_d, table1, t2s_d, table2, out_d = (
        env["x_d"], env["t1s_d"], env["table1"], env["t2s_d"], env["table2"],
        env["out_d"])
    wp, gp, dp, pp, np_ = env["wp"], env["gp"], env["dp"], env["pp"], env["np_"]
    op_ = env["op_"]
    ident, iota, wc1, wc2, b1r, b2r = (
        env["ident"], env["iota"], env["wc1"], env["wc2"], env["b1r"],
        env["b2r"])
    esrc, edst, esrc2, edst2, edstl = (
        env["esrc"], env["edst"], env["esrc2"], env["edst2"], env["edstl"])
    h2big = env["h2big"]
    if debug_tabs:
        dbg1_d, dbg2_d = env["dbg1_d"], env["dbg2_d"]

    if True:
        if True:
            # ---------- node phase, layer 1 ----------
            for t in range(nt):
                xt = wp.tile([P, IN_DIM], f32, tag="xt")
                nc.sync.dma_start(out=xt[:, :], in_=x_d[t * P:(t + 1) * P, :])
                tp = np_.tile([IN_DIM, P], f32, tag="tps")
                nc.tensor.transpose(tp[:, :], xt[:, :], ident[:, :])
                xT = wp.tile([IN_DIM, P], f32, tag="xT")
                nc.vector.tensor_copy(out=xT[:, :], in_=tp[:, :])
                hp = np_.tile([P, 80], f32, tag="hps")
                nc.tensor.matmul(out=hp[:, :], lhsT=xT[:, :], rhs=wc1[:, :],
                                 start=True, stop=True)
                ht = wp.tile([P, 80], f32, tag="ht")
                nc.vector.tensor_copy(out=ht[:, :], in_=hp[:, :])
                nc.sync.dma_start(out=t1s_d[t * P:(t + 1) * P, :], in_=ht[:, :])

            nc.gpsimd.collective_compute(
                "AllGather", Alu.bypass, replica_groups=groups,
                ins=[t1s_d[:, :]], outs=[table1[:, :]])
            if debug_tabs:
                for t in range(n_pad // P):
                    dt_ = wp.tile([P, 80], f32, tag="dbg")
                    nc.sync.dma_start(out=dt_[:, :], in_=table1[t*P:(t+1)*P, :])
                    nc.sync.dma_start(out=dbg1_d[t*P:(t+1)*P, :], in_=dt_[:, :])

            # ---------- edge phases ----------
            import os as _os
            abl = _os.environ.get("K_ABLATE", "")

            def edge_phase(table, RL, GW, H, src_t, dstg_t, layer):
                SO = 64          # score col offset within gathered row
                for (sl0, nsl, bb0, nblk) in supers:
                    G = gp.tile([P, nblk * GW], f32, tag="G")
                    sD = gp.tile([P, nblk * H], f32, tag="sD")
                    if abl in ("nogather", "novec"):
                        pass
                    else:
                        for j in range(nblk):
                            nc.gpsimd.indirect_dma_start(
                                out=G[:, j * GW:(j + 1) * GW], out_offset=None,
                                in_=table[:, :],
                                in_offset=IndirectOffsetOnAxis(
                                    ap=src_t[:, bb0 + j:bb0 + j + 1], axis=0))
                            if abl == "nosd":
                                continue
                            nc.gpsimd.indirect_dma_start(
                                out=sD[:, j * H:(j + 1) * H], out_offset=None,
                                in_=table[:, :],
                                in_offset=IndirectOffsetOnAxis(
                                    ap=dstg_t[:, bb0 + j:bb0 + j + 1], axis=0),
                                element_offset=64 + H)
                    if abl == "gathersonly":
                        continue
                    if abl == "novec":
                        # skip all vector/scalar edge math; matmuls read zeros
                        bb = bb0
                        oh = gp.tile([P, nblk * W], f32, tag="oh")
                        nc.vector.memset(oh[:, :], 0.0)
                        for s in range(sl0, sl0 + nsl):
                            K = Ks[s]
                            ps = pp.tile([W, GW], f32, tag="ps")
                            for j in range(K):
                                jj = bb - bb0 + j
                                nc.tensor.matmul(
                                    out=ps[:, :],
                                    lhsT=oh[:, jj * W:(jj + 1) * W],
                                    rhs=G[:, jj * GW:(jj + 1) * GW],
                                    start=(j == 0), stop=(j == K - 1))
                            bb += K
                            ot = dp.tile([W, 64], f32, tag="ot")
                            nc.vector.tensor_copy(out=ot[:, :], in_=ps[:, :64])
                            if layer == 1:
                                pq, pr = (s % 2) * W, (s // 2) * W
                                nc.vector.tensor_copy(
                                    out=h2big[pq:pq + W, pr:pr + W], in_=ot[:, :])
                            else:
                                nc.sync.dma_start(
                                    out=out_d[s * W:(s + 1) * W, :], in_=ot[:, :])
                        return
                    # e = sS + sD ; lrelu ; p = exp -> back into G score cols
                    e = wp.tile([P, nblk * H], f32, tag="e")
                    nc.vector.tensor_tensor(
                        out=_sub(e[:, :], 0, [[H, nblk], [1, H]]),
                        in0=_sub(G[:, :], SO, [[GW, nblk], [1, H]]),
                        in1=_sub(sD[:, :], 0, [[H, nblk], [1, H]]),
                        op=Alu.add)
                    nc.vector.scalar_tensor_tensor(
                        out=e[:, :], in0=e[:, :], scalar=NEG_SLOPE,
                        in1=e[:, :], op0=Alu.mult, op1=Alu.max)
                    nc.scalar.activation(
                        out=_sub(G[:, :], SO, [[GW, nblk], [1, H]]),
                        in_=_sub(e[:, :], 0, [[H, nblk], [1, H]]),
                        func=Act.Exp)
                    # onehot[e, d] = (dstl[e] == d)
                    oh = op_.tile([P, nblk * W], f32, tag="oh")
                    nc.vector.tensor_tensor(
                        out=_sub(oh[:, :], 0, [[W, nblk], [1, W]]),
                        in0=_sub(iota[:, :], 0, [[0, nblk], [1, W]]),
                        in1=_sub(edstl[:, :], bb0, [[1, nblk], [0, W]]),
                        op=Alu.is_equal)
                    # msg = h * p (per-head broadcast), in place on G h-cols
                    if H == 1:
                        in1p = _sub(G[:, :], SO, [[GW, nblk], [1, 1], [0, 64]])
                        in0m = _sub(G[:, :], 0, [[GW, nblk], [64, 1], [1, 64]])
                    else:
                        in1p = _sub(G[:, :], SO, [[GW, nblk], [1, H], [0, 64 // H]])
                        in0m = _sub(G[:, :], 0, [[GW, nblk], [64 // H, H], [1, 64 // H]])
                    nc.vector.tensor_tensor(out=in0m, in0=in0m, in1=in1p,
                                            op=Alu.mult)
                    # per-slot scatter matmuls + drain
                    bb = bb0
                    for s in range(sl0, sl0 + nsl):
                        K = Ks[s]
                        ps = pp.tile([W, GW], f32, tag="ps")
                        for j in range(K):
                            jj = bb - bb0 + j
                            nc.tensor.matmul(
                                out=ps[:, :],
                                lhsT=oh[:, jj * W:(jj + 1) * W],
                                rhs=G[:, jj * GW:(jj + 1) * GW],
                                start=(j == 0), stop=(j == K - 1))
                        bb += K
                        den = dp.tile([W, H], f32, tag="den")
                        nc.vector.tensor_scalar_add(den[:, :], ps[:, 64:64 + H],
                                                    1e-10)
                        inv = dp.tile([W, H], f32, tag="inv")
                        nc.vector.reciprocal(inv[:, :], den[:, :])
                        ot = dp.tile([W, 64], f32, tag="ot")
                        if H == 1:
                            o_ap = _sub(ot[:, :], 0, [[64, 1], [1, 64]])
                            s_ap = _sub(ps[:, :], 0, [[64, 1], [1, 64]])
                            i_ap = _sub(inv[:, :], 0, [[1, 1], [0, 64]])
                        else:
                            o_ap = _sub(ot[:, :], 0, [[64 // H, H], [1, 64 // H]])
                            s_ap = _sub(ps[:, :], 0, [[64 // H, H], [1, 64 // H]])
                            i_ap = _sub(inv[:, :], 0, [[1, H], [0, 64 // H]])
                        nc.vector.tensor_tensor(out=o_ap, in0=s_ap, in1=i_ap,
                                                op=Alu.mult)
                        if layer == 1:
                            nc.vector.tensor_tensor(out=ot[:, :], in0=ot[:, :],
                                                    in1=b1r[:, :], op=Alu.add)
                            ex = dp.tile([W, 64], f32, tag="ex")
                            nc.scalar.activation(out=ex[:, :], in_=ot[:, :],
                                                 func=Act.Exp)
                            nc.vector.tensor_scalar(
                                out=ex[:, :], in0=ex[:, :], scalar1=-1.0,
                                scalar2=0.0, op0=Alu.add, op1=Alu.min)
                            rl = dp.tile([W, 64], f32, tag="rl")
                            nc.vector.tensor_scalar_max(rl[:, :], ot[:, :], 0.0)
                            pq, pr = (s % 2) * W, (s // 2) * W
                            nc.vector.tensor_tensor(
                                out=h2big[pq:pq + W, pr:pr + W],
                                in0=ex[:, :], in1=rl[:, :], op=Alu.add)
                        else:
                            ob = dp.tile([W, 64], f32, tag="ob")
                            nc.vector.tensor_tensor(out=ob[:, :], in0=ot[:, :],
                                                    in1=b2r[:, :], op=Alu.add)
                            nc.sync.dma_start(
                                out=out_d[s * W:(s + 1) * W, :], in_=ob[:, :])

            edge_phase(table1, 80, 72, HEADS1, esrc, edst, layer=1)

            # ---------- node phase, layer 2 (from SBUF h2big) ----------
            for t in range(nt):
                tp2 = np_.tile([64, P], f32, tag="tps")
                nc.tensor.transpose(tp2[:, :], h2big[:, t * 64:(t + 1) * 64],
                                    ident[:, :])
                h2T = wp.tile([64, P], f32, tag="h2T")
                nc.vector.tensor_copy(out=h2T[:, :], in_=tp2[:, :])
                hp2 = np_.tile([P, 66], f32, tag="hps")
                nc.tensor.matmul(out=hp2[:, :], lhsT=h2T[:, :], rhs=wc2[:, :],
                                 start=True, stop=True)
                h2t = wp.tile([P, 66], f32, tag="ht")
                nc.vector.tensor_copy(out=h2t[:, :], in_=hp2[:, :])
                nc.sync.dma_start(out=t2s_d[t * P:(t + 1) * P, :], in_=h2t[:, :])

            nc.gpsimd.collective_compute(
                "AllGather", Alu.bypass, replica_groups=groups,
                ins=[t2s_d[:, :]], outs=[table2[:, :]])
            if debug_tabs:
                for t in range(n_pad // P):
                    dt2_ = wp.tile([P, 66], f32, tag="dbg")
                    nc.sync.dma_start(out=dt2_[:, :], in_=table2[t*P:(t+1)*P, :])
                    nc.sync.dma_start(out=dbg2_d[t*P:(t+1)*P, :], in_=dt2_[:, :])

            edge_phase(table2, 66, 65, 1, esrc2, edst2, layer=2)


def kernel(**inputs):
    import concourse.bacc as bacc
    from concourse.bass_utils import run_bass_kernel_spmd

    n = inputs["x"].shape[0]
    cfg, in_maps, perm = _prep(**inputs)

    nc = bacc.Bacc("TRN2", target_bir_lowering=False, debug=False,
                   num_devices=cfg["n_cores"])
    _build(nc, cfg)
    nc.compile()

    res = run_bass_kernel_spmd(nc, in_maps,
                               core_ids=list(range(cfg["n_cores"])))
    full = np.concatenate([r["out"] for r in res.results], axis=0)
    out = full[perm[:n]]
    return np.ascontiguousarray(out, np.float32)



# revision 25
# speedup vs baseline: 360.5021x; 360.5021x over previous
"""2-layer GAT (GATConv x2, PyG-style) on 8 Trainium2 NeuronCores — v2.

Strategy (edge-parallel, dst-sharded, slot-ordered, bf16 tables):
  - Nodes padded to 50176 and sharded contiguously: core c owns 6272 nodes
    (98 windows x 64 dst). Host permutes nodes into per-core slot order
    (windows sorted by edge count) so all cores run one SPMD program; the
    host un-permutes the output. Both layers share the same ordering, so
    one gather-index stream serves both.
  - Edges (incl. self loops) are bucketed by dst window; within a window
    they are split into "lo" blocks (src row < 25088) and "hi" blocks so
    each 128-edge block is src-half homogeneous. Bulk gathers use
    dma_gather (int16 indices, 256B rows): one lo + one hi gather per
    super-block of ~64 edge blocks instead of per-block indirect DMAs.
  - Table rows are 128 bf16 (256B): cols 0:64 h (bf16), cols 64:64+2*H the
    per-src attention score sS as raw f32 bytes, rest zero. The per-dst
    score sD never travels through DRAM: it is broadcast to edges with a
    small PE matmul (transposed one-hot x sdw) per block.
  - p = exp(leakyrelu(sS+sD)) (scores are bounded; no segment-max needed),
    messages m = h*p accumulate per dst via one-hot matmuls into PSUM.
    Layer-1 scatters transposed ([feat, dst]) so the normalized+ELU result
    lands directly as the lhsT of the layer-2 node matmul; layer-2
    scatters dst-major and stores straight to the output.
  - Softmax normalization + bias + activation are drained in a few big
    batched ops over a staging buffer, not per window.
"""

import numpy as np
import ml_dtypes

BF = ml_dtypes.bfloat16

P = 128          # edges per block / SBUF partitions
W = 64           # dst nodes per window
NC = 8           # cores
WPC = 98         # windows per core
NPC = WPC * W    # nodes per core (6272)
NP = NC * NPC    # padded node count (50176)
HALF = NP // 2   # 25088: table half split for int16 gather indices
IN_DIM = 128
H1 = 8           # layer-1 heads
OUT_DIM = 64
NEG_SLOPE = 0.2
RLB = 128        # table row length in bf16 elems (256 bytes)
SUPER_BLK = 32   # max edge blocks per super
DCH = 8          # slots per drain chunk


def _mk_head_mat(a):
    """[H, C] attention vector -> [H*C, H] block-diagonal matrix."""
    H, C = a.shape
    A = np.zeros((H * C, H), np.float32)
    for h in range(H):
        A[h * C:(h + 1) * C, h] = a[h]
    return A


def _prep(x, edge_index, W1, a_src1, a_dst1, b1, W2, a_src2, a_dst2, b2,
          n_cores=NC, wpc=WPC):
    """Host-side preprocessing. Returns (cfg, in_maps, perm)."""
    npc = wpc * W
    n_pad = n_cores * npc
    n = x.shape[0]
    assert n <= n_pad

    x = np.asarray(x, np.float32)
    xp = np.zeros((n_pad, IN_DIM), np.float32)
    xp[:n] = x

    ei = np.asarray(edge_index)
    src = np.concatenate([ei[0], np.arange(n)]).astype(np.int64)
    dst = np.concatenate([ei[1], np.arange(n)]).astype(np.int64)

    win = (dst // W).astype(np.int64)
    half = (src >= HALF).astype(np.int64)
    # sort edges by (window, half); stable for determinism
    order = np.lexsort((half, win))
    src, dst, win, half = src[order], dst[order], win[order], half[order]
    nw = n_pad // W
    # counts per (window, half)
    key = win * 2 + half
    counts2 = np.bincount(key, minlength=nw * 2).reshape(nw, 2)
    starts2 = np.concatenate([[0], np.cumsum(counts2.reshape(-1))]).reshape(-1)

    counts_c = counts2.reshape(n_cores, wpc, 2)
    Klo_c = np.ceil(counts_c[:, :, 0] / P).astype(np.int64)
    Khi_c = np.ceil(counts_c[:, :, 1] / P).astype(np.int64)
    tot_c = counts_c.sum(axis=2)
    orders = [np.argsort(-tot_c[c], kind="stable") for c in range(n_cores)]
    Klo = np.max(np.stack([Klo_c[c][orders[c]] for c in range(n_cores)]), axis=0)
    Khi = np.max(np.stack([Khi_c[c][orders[c]] for c in range(n_cores)]), axis=0)
    # every window has self-loops so Klo+Khi >= 1 automatically
    assert (Klo + Khi).min() >= 1
    Mtot = int(Klo.sum() + Khi.sum())

    # perm[node] = global row in the slot-ordered table (same for both layers)
    perm = np.empty(n_pad, np.int64)
    for c in range(n_cores):
        inv = np.empty(wpc, np.int64)
        inv[orders[c]] = np.arange(wpc)
        wl = np.arange(wpc)
        base = (c * wpc + wl) * W
        for woff in range(W):
            perm[base + woff] = c * npc + inv * W + woff
    perm_inv = np.empty(n_pad, np.int64)
    perm_inv[perm] = np.arange(n_pad)

    # supers: greedy grouping of slots by block budget; within a super the
    # lo blocks of all slots come first, then the hi blocks.
    supers = []   # (sl0, nsl, bb0, nlo, nhi)
    block_slot = []   # global block id -> slot
    super_js = []     # per super: {slot: [local j list]}
    s = 0
    bb0 = 0
    while s < wpc:
        s0 = s
        nb = 0
        while s < wpc and nb + Klo[s] + Khi[s] <= SUPER_BLK:
            nb += int(Klo[s] + Khi[s])
            s += 1
        nsl = s - s0
        nlo = int(Klo[s0:s].sum())
        nhi = int(Khi[s0:s].sum())
        js = {}
        j = 0
        for sl in range(s0, s):
            js[sl] = list(range(j, j + int(Klo[sl])))
            j += int(Klo[sl])
        for sl in range(s0, s):
            js[sl] = js[sl] + list(range(j, j + int(Khi[sl])))
            j += int(Khi[sl])
        assert j == nlo + nhi == nb
        for sl in range(s0, s):
            for _ in range(int(Klo[sl])):
                block_slot.append(sl)
        for sl in range(s0, s):
            for _ in range(int(Khi[sl])):
                block_slot.append(sl)
        supers.append((s0, nsl, bb0, nlo, nhi))
        super_js.append(js)
        bb0 += nb
    assert bb0 == Mtot

    # per-core packed streams
    in_maps = []
    for c in range(n_cores):
        idx_vals = np.zeros((Mtot, P), np.int64)    # gather index per edge
        dstl_vals = np.full((Mtot, P), W, np.int64)  # local dst (W = pad)
        for si, (s0, nsl, b0, nlo, nhi) in enumerate(supers):
            js = super_js[si]
            for sl in range(s0, s0 + nsl):
                wloc = orders[c][sl]
                wglob = c * wpc + wloc
                for hf, Kh in ((0, Klo), (1, Khi)):
                    e0 = starts2[wglob * 2 + hf]
                    e1 = starts2[wglob * 2 + hf + 1]
                    ew = np.arange(e0, e1)
                    jlist = js[sl][:int(Klo[sl])] if hf == 0 else js[sl][int(Klo[sl]):]
                    for k, jl in enumerate(jlist):
                        blk = b0 + jl
                        seg = ew[k * P:(k + 1) * P]
                        m = seg.size
                        if m:
                            pv = perm[src[seg]]
                            idx_vals[blk, :m] = pv - (HALF if hf else 0)
                            dstl_vals[blk, :m] = dst[seg] - wglob * W
        # int16 wrapped index pack: edge (p, blk) -> [p%16 (+16g), blk*8 + p//16]
        iv = idx_vals.reshape(Mtot, 8, 16)           # [blk, q, r]
        idx16 = np.zeros((128, Mtot * 8), np.int16)
        base = np.transpose(iv, (2, 0, 1)).reshape(16, Mtot * 8)
        for g in range(8):
            idx16[g * 16:(g + 1) * 16, :] = base
        dstl = np.ascontiguousarray(dstl_vals.T, np.float32)   # [P, Mtot]
        dstlT = np.ascontiguousarray(dstl_vals, BF)            # [Mtot, P]

        xs = xp[perm_inv[c * npc:(c + 1) * npc]]               # slot-ordered
        xT = np.ascontiguousarray(xs.T, np.float32)            # [128, npc]

        in_maps.append({
            "xT": xT,
            "idx": np.ascontiguousarray(idx16),
            "dstl": dstl,
            "dstlT": dstlT,
        })

    W1 = np.asarray(W1, np.float32)
    W2 = np.asarray(W2, np.float32)
    wc1 = np.concatenate(
        [W1, W1 @ _mk_head_mat(np.asarray(a_src1, np.float32)),
         W1 @ _mk_head_mat(np.asarray(a_dst1, np.float32))], axis=1)  # [128, 80]
    wc2 = np.concatenate(
        [W2, W2 @ np.asarray(a_src2, np.float32).T,
         W2 @ np.asarray(a_dst2, np.float32).T], axis=1)              # [64, 66]
    E8 = np.zeros((H1, W), np.float32)
    for h in range(H1):
        E8[h, h * 8:(h + 1) * 8] = 1.0
    b1c = np.asarray(b1, np.float32).reshape(W, 1)
    b2r = np.tile(np.asarray(b2, np.float32)[None, :], (W, 1))
    for m in in_maps:
        m["wc1"] = np.ascontiguousarray(wc1, np.float32)
        m["wc2"] = np.ascontiguousarray(wc2, np.float32)
        m["E8"] = np.ascontiguousarray(E8, np.float32)
        m["b1c"] = np.ascontiguousarray(b1c, np.float32)
        m["b2r"] = np.ascontiguousarray(b2r, np.float32)

    cfg = dict(n_cores=n_cores, wpc=wpc, npc=npc, n_pad=n_pad,
               supers=supers, super_js=super_js,
               block_slot=block_slot, Mtot=Mtot,
               Klo=[int(k) for k in Klo], Khi=[int(k) for k in Khi])
    return cfg, in_maps, perm


def _ap(base, off, dims):
    """Custom multi-level free-dim AP on top of a tile's [:, :] AP."""
    import concourse.bass as bass
    return bass.AP(tensor=base.tensor, offset=base.offset + off,
                   ap=[list(base.ap[0])] + [list(d) for d in dims])


def _build(nc, cfg, reps=1):
    import concourse.mybir as mybir
    import concourse.tile as tile
    from concourse.library_config import mlp

    f32 = mybir.dt.float32
    bf16 = mybir.dt.bfloat16
    i16 = mybir.dt.int16

    n_cores, wpc, npc, n_pad = cfg["n_cores"], cfg["wpc"], cfg["npc"], cfg["n_pad"]
    Mtot = cfg["Mtot"]
    nt = npc // P

    xT_d = nc.dram_tensor("xT", [P, npc], f32, kind="ExternalInput")
    idx_d = nc.dram_tensor("idx", [P, Mtot * 8], i16, kind="ExternalInput")
    dstl_d = nc.dram_tensor("dstl", [P, Mtot], f32, kind="ExternalInput")
    dstlT_d = nc.dram_tensor("dstlT", [Mtot, P], bf16, kind="ExternalInput")
    wc1_d = nc.dram_tensor("wc1", [IN_DIM, 80], f32, kind="ExternalInput")
    wc2_d = nc.dram_tensor("wc2", [W, 66], f32, kind="ExternalInput")
    E8_d = nc.dram_tensor("E8", [H1, W], f32, kind="ExternalInput")
    b1c_d = nc.dram_tensor("b1c", [W, 1], f32, kind="ExternalInput")
    b2r_d = nc.dram_tensor("b2r", [W, OUT_DIM], f32, kind="ExternalInput")
    out_d = nc.dram_tensor("out", [npc, OUT_DIM], f32, kind="ExternalOutput")

    t1h_d = nc.dram_tensor("t1h", [npc, RLB], bf16, kind="Internal")
    tab1_d = nc.dram_tensor("tab1", [n_pad, RLB], bf16, kind="Internal")
    t2h_d = nc.dram_tensor("t2h", [npc, RLB], bf16, kind="Internal")
    tab2_d = nc.dram_tensor("tab2", [n_pad, RLB], bf16, kind="Internal")

    iota64_d = nc.inline_tensor(
        np.arange(W, dtype=BF)[:, None], "iota64")
    iotaw_d = nc.inline_tensor(
        np.tile(np.arange(W, dtype=np.float32), (P, 1)), "iotaw")

    with tile.TileContext(nc) as tc:
        with tc.tile_pool(name="const", bufs=1) as cp, \
             tc.tile_pool(name="work", bufs=3) as wp, \
             tc.tile_pool(name="drain", bufs=2) as dp, \
             tc.tile_pool(name="gath", bufs=2) as gp, \
             tc.tile_pool(name="ohp", bufs=2) as op_, \
             tc.tile_pool(name="ohtp", bufs=2) as otp, \
             tc.tile_pool(name="dtp", bufs=2) as dtp, \
             tc.tile_pool(name="nps", bufs=2, space="PSUM") as np_, \
             tc.tile_pool(name="sps", bufs=2, space="PSUM") as pp, \
             tc.tile_pool(name="sdp", bufs=2, space="PSUM") as sp:

            nc.gpsimd.load_library(mlp)

            def cload(name, shape, dt, src):
                t = cp.tile(shape, dt, tag=name)
                nc.sync.dma_start(out=t[:, :], in_=src[:, :])
                return t

            idx = cload("idx", [P, Mtot * 8], i16, idx_d)
            dstl = cload("dstl", [P, Mtot], f32, dstl_d)
            wc1 = cload("wc1", [IN_DIM, 80], f32, wc1_d)
            wc2 = cload("wc2", [W, 66], f32, wc2_d)
            E8 = cload("E8", [H1, W], f32, E8_d)
            b1c = cload("b1c", [W, 1], f32, b1c_d)
            b2r = cload("b2r", [W, OUT_DIM], f32, b2r_d)
            iota64 = cload("iota64", [W, 1], bf16, iota64_d)
            iotaw = cload("iotaw", [P, W], f32, iotaw_d)

            # sD tables in bf16 double-double: [hi | lo] per slot
            sdw1 = cp.tile([W, wpc * 2 * H1], bf16, tag="sdw1")
            sdw2 = cp.tile([W, wpc * 2], bf16, tag="sdw2")
            h2T = cp.tile([W, npc], f32, tag="h2T")

            env = dict(locals())
            for _rep in range(reps):
                _body(nc, cfg, env)


def _body(nc, cfg, env):
    import concourse.bass as bass
    import concourse.mybir as mybir

    f32 = mybir.dt.float32
    bf16 = mybir.dt.bfloat16
    Alu = mybir.AluOpType
    Act = mybir.ActivationFunctionType

    n_cores, wpc, npc, n_pad = cfg["n_cores"], cfg["wpc"], cfg["npc"], cfg["n_pad"]
    supers, super_js = cfg["supers"], cfg["super_js"]
    block_slot, Klo = cfg["block_slot"], cfg["Klo"]
    nt = npc // P
    groups = [list(range(n_cores))]

    wp, gp, op_, otp = env["wp"], env["gp"], env["op_"], env["otp"]
    np_, pp, sp, cp = env["np_"], env["pp"], env["sp"], env["cp"]
    dp, dtp = env["dp"], env["dtp"]
    idx, dstl, wc1, wc2 = env["idx"], env["dstl"], env["wc1"], env["wc2"]
    E8, b1c, b2r, iota64, iotaw = (env["E8"], env["b1c"], env["b2r"],
                                   env["iota64"], env["iotaw"])
    sdw1, sdw2, h2T = env["sdw1"], env["sdw2"], env["h2T"]
    xT_d, idx_d, dstl_d, dstlT_d = (env["xT_d"], env["idx_d"], env["dstl_d"],
                                    env["dstlT_d"])
    t1h_d, tab1_d, t2h_d, tab2_d = (env["t1h_d"], env["tab1_d"], env["t2h_d"],
                                    env["tab2_d"])
    out_d = env["out_d"]

    # ---------------- layer-1 node phase ----------------
    for t in range(nt):
        xt = wp.tile([P, P], f32, tag="xt")
        nc.sync.dma_start(out=xt[:, :], in_=xT_d[:, t * P:(t + 1) * P])
        hp = np_.tile([P, 80], f32, tag="hp")
        nc.tensor.matmul(out=hp[:, :], lhsT=xt[:, :],
                         rhs=wc1[:, :], start=True, stop=True)
        st = wp.tile([P, RLB], bf16, tag="st")
        nc.vector.tensor_copy(out=st[:, :W], in_=hp[:, :W])
        stf = st[:, :].bitcast(f32)
        nc.vector.tensor_copy(
            out=bass.AP(tensor=stf.tensor, offset=stf.offset + 32,
                        ap=[list(stf.ap[0])] + [[1, H1]]),
            in_=hp[:, W:W + H1])
        nc.vector.memset(st[:, W + 2 * H1:], 0.0)
        for wi, rows in ((2 * t, slice(0, W)), (2 * t + 1, slice(W, P))):
            hi = sdw1[:, wi * 2 * H1:wi * 2 * H1 + H1]
            nc.vector.tensor_copy(out=hi, in_=hp[rows, 72:80])
            hiF = wp.tile([W, H1], f32, tag="hiF")
            nc.vector.tensor_copy(out=hiF[:, :], in_=hi)
            res = wp.tile([W, H1], f32, tag="res")
            nc.vector.tensor_tensor(out=res[:, :], in0=hp[rows, 72:80],
                                    in1=hiF[:, :], op=Alu.subtract)
            nc.vector.tensor_copy(
                out=sdw1[:, wi * 2 * H1 + H1:(wi + 1) * 2 * H1],
                in_=res[:, :])
        nc.sync.dma_start(out=t1h_d[t * P:(t + 1) * P, :], in_=st[:, :])

    nc.gpsimd.collective_compute(
        "AllGather", Alu.bypass, replica_groups=groups,
        ins=[t1h_d[:, :]], outs=[tab1_d[:, :]])

    # ---------------- layer-1 drain chunk (softmax div + bias + ELU) ----
    def flush1(chunk, ck, c):
        c0 = c * DCH * W
        den = dp.tile([H1, DCH * W], f32, tag="den")
        nc.vector.tensor_scalar_add(den[:, :ck], chunk[64:72, :ck], 1e-10)
        inv = dp.tile([H1, DCH * W], f32, tag="inv")
        nc.vector.reciprocal(inv[:, :ck], den[:, :ck])
        pb = np_.tile([W, DCH * W], f32, tag="pb")
        nc.tensor.matmul(out=pb[:, :ck], lhsT=E8[:, :], rhs=inv[:, :ck],
                         start=True, stop=True)
        ot = dp.tile([W, DCH * W], f32, tag="ot")
        nc.vector.tensor_tensor(out=ot[:, :ck], in0=chunk[:64, :ck],
                                in1=pb[:, :ck], op=Alu.mult)
        nc.vector.tensor_tensor(out=ot[:, :ck], in0=ot[:, :ck],
                                in1=b1c[:, :1].to_broadcast([W, ck]),
                                op=Alu.add)
        ex = dp.tile([W, DCH * W], f32, tag="ex")
        nc.scalar.activation(out=ex[:, :ck], in_=ot[:, :ck], func=Act.Exp)
        nc.vector.tensor_scalar(out=ex[:, :ck], in0=ex[:, :ck], scalar1=-1.0,
                                scalar2=0.0, op0=Alu.add, op1=Alu.min)
        rl = dp.tile([W, DCH * W], f32, tag="rl")
        nc.vector.tensor_scalar_max(rl[:, :ck], ot[:, :ck], 0.0)
        nc.vector.tensor_tensor(out=h2T[:, c0:c0 + ck], in0=ex[:, :ck],
                                in1=rl[:, :ck], op=Alu.add)

    # ---------------- layer-2 drain chunk (softmax div + bias + store) ----
    def flush2(chunk, ck, c):
        ns = ck // 65
        den2 = dp.tile([W, DCH], f32, tag="den2")
        nc.vector.tensor_scalar_add(
            den2[:, :ns], _ap(chunk[:, :], 64, [[65, ns]]), 1e-10)
        inv2 = dp.tile([W, DCH], f32, tag="inv2")
        nc.vector.reciprocal(inv2[:, :ns], den2[:, :ns])
        ob = dp.tile([W, DCH * OUT_DIM], f32, tag="ob")
        nc.vector.tensor_tensor(
            out=_ap(ob[:, :], 0, [[OUT_DIM, ns], [1, OUT_DIM]]),
            in0=_ap(chunk[:, :], 0, [[65, ns], [1, OUT_DIM]]),
            in1=_ap(inv2[:, :], 0, [[1, ns], [0, OUT_DIM]]),
            op=Alu.mult)
        nc.vector.tensor_tensor(
            out=_ap(ob[:, :], 0, [[OUT_DIM, ns], [1, OUT_DIM]]),
            in0=_ap(ob[:, :], 0, [[OUT_DIM, ns], [1, OUT_DIM]]),
            in1=_ap(b2r[:, :], 0, [[0, ns], [1, OUT_DIM]]),
            op=Alu.add)
        nc.sync.dma_start(
            out=bass.AP(tensor=out_d[:, :].tensor,
                        offset=c * DCH * W * OUT_DIM,
                        ap=[[OUT_DIM, W], [W * OUT_DIM, ns], [1, OUT_DIM]]),
            in_=ob[:, :ns * OUT_DIM])

    # ---------------- edge phase ----------------
    def edge_phase(layer, tab, H, GWm, sdw, scol, flush):
        chunk = None
        for si, (sl0, nsl, bb0, nlo, nhi) in enumerate(supers):
            nblk = nlo + nhi
            js = super_js[si]
            G = gp.tile([P, nblk * RLB], bf16, tag="G")

            def out3(apx, k):
                return bass.AP(tensor=apx.tensor, offset=apx.offset,
                               ap=[list(apx.ap[0])] + [[RLB, k], [1, RLB]])

            # chunk gathers to <=1024 indices (SWDGE descriptor-ring limit)
            GCH = 8

            def gathers(col0, nb, tab_slice):
                for off in range(0, nb, GCH):
                    k = min(GCH, nb - off)
                    a = col0 + off
                    nc.gpsimd.dma_gather(
                        out_ap=out3(G[:, a * RLB:(a + k) * RLB], k),
                        in_ap=tab_slice,
                        idxs_ap=idx[:, (bb0 + a) * 8:(bb0 + a + k) * 8],
                        num_idxs=k * P, num_idxs_reg=k * P, elem_size=RLB)

            if nlo:
                gathers(0, nlo, tab[0:HALF, :])
            if nhi:
                gathers(nlo, nhi, tab[HALF:, :])

            dT = dtp.tile([1, nblk * P], bf16, tag="dT")
            nc.sync.dma_start(out=dT[:, :], in_=dstlT_d[bb0:bb0 + nblk, :])
            ohT = otp.tile([W, nblk * P], bf16, tag="ohT")
            nc.gpsimd.partition_broadcast(ohT[:, :], dT[:1, :], channels=W)
            nc.vector.tensor_tensor(
                out=ohT[:, :],
                in0=iota64[:, :1].to_broadcast([W, nblk * P]),
                in1=ohT[:, :], op=Alu.is_equal)

            oh = op_.tile([P, nblk * W], bf16, tag="oh")
            nc.vector.tensor_tensor(
                out=_ap(oh[:, :], 0, [[W, nblk], [1, W]]),
                in0=_ap(iotaw[:, :], 0, [[0, nblk], [1, W]]),
                in1=_ap(dstl[:, :], bb0, [[1, nblk], [0, W]]),
                op=Alu.is_equal)

            # per-edge sD via transposed-one-hot matmuls ([hi | lo] rhs)
            psD = sp.tile([P, nblk * 2 * H], f32, tag="psD")
            for j in range(nblk):
                s = block_slot[bb0 + j]
                nc.tensor.matmul(
                    out=psD[:, j * 2 * H:(j + 1) * 2 * H],
                    lhsT=ohT[:, j * P:(j + 1) * P],
                    rhs=sdw[:, s * 2 * H:(s + 1) * 2 * H],
                    start=True, stop=True)

            # e = sS + sD_hi + sD_lo ; lrelu ; p = exp -> G score cols (bf16)
            Gf = G[:, :].bitcast(f32)
            sS = bass.AP(tensor=Gf.tensor, offset=Gf.offset + 32,
                         ap=[list(Gf.ap[0])] + [[RLB // 2, nblk], [1, H]])
            eS = wp.tile([P, nblk * H], f32, tag="eS")
            nc.vector.tensor_tensor(
                out=eS[:, :], in0=sS,
                in1=_ap(psD[:, :], 0, [[2 * H, nblk], [1, H]]), op=Alu.add)
            nc.vector.tensor_tensor(
                out=eS[:, :], in0=eS[:, :],
                in1=_ap(psD[:, :], H, [[2 * H, nblk], [1, H]]), op=Alu.add)
            nc.vector.scalar_tensor_tensor(
                out=eS[:, :], in0=eS[:, :], scalar=NEG_SLOPE,
                in1=eS[:, :], op0=Alu.mult, op1=Alu.max)
            p_dst = _ap(G[:, :], W, [[RLB, nblk], [1, H]])
            nc.scalar.activation(out=p_dst, in_=eS[:, :], func=Act.Exp)

            # msg = h * p (per-head broadcast)
            if H == 1:
                m_ap = _ap(G[:, :], 0, [[RLB, nblk], [1, W]])
                p_ap = _ap(G[:, :], W, [[RLB, nblk], [0, W]])
            else:
                m_ap = _ap(G[:, :], 0, [[RLB, nblk], [H, H], [1, 64 // H]])
                p_ap = _ap(G[:, :], W, [[RLB, nblk], [1, H], [0, 64 // H]])
            nc.vector.tensor_tensor(out=m_ap, in0=m_ap, in1=p_ap, op=Alu.mult)

            # scatter per slot; stage into drain chunks of DCH slots
            for s in range(sl0, sl0 + nsl):
                jl = js[s]
                if layer == 1:
                    ps = pp.tile([64 + H, W], f32, tag="ps")
                    for k, j in enumerate(jl):
                        nc.tensor.matmul(
                            out=ps[:, :],
                            lhsT=_ap(G[:, :], j * RLB, [[1, GWm]]),
                            rhs=oh[:, j * W:(j + 1) * W],
                            start=(k == 0), stop=(k == len(jl) - 1))
                    rows = 64 + H
                else:
                    ps = pp.tile([W, GWm], f32, tag="ps")
                    for k, j in enumerate(jl):
                        nc.tensor.matmul(
                            out=ps[:, :],
                            lhsT=oh[:, j * W:(j + 1) * W],
                            rhs=_ap(G[:, :], j * RLB, [[1, GWm]]),
                            start=(k == 0), stop=(k == len(jl) - 1))
                    rows = W
                if chunk is None:
                    chunk = dp.tile([rows, DCH * scol], f32,
                                    tag=f"stage{layer}")
                sloc = s % DCH
                nc.vector.tensor_copy(
                    out=chunk[:, sloc * scol:(sloc + 1) * scol], in_=ps[:, :])
                if sloc == DCH - 1 or s == wpc - 1:
                    flush(chunk, (sloc + 1) * scol, s // DCH)
                    chunk = None

    edge_phase(1, tab1_d, H1, 64 + H1, sdw1, W, flush1)

    # ---------------- layer-2 node phase ----------------
    for t in range(nt):
        hp2 = np_.tile([P, 80], f32, tag="hp")
        nc.tensor.matmul(out=hp2[:, :66], lhsT=h2T[:, t * P:(t + 1) * P],
                         rhs=wc2[:, :], start=True, stop=True)
        st2 = wp.tile([P, RLB], bf16, tag="st2")
        nc.vector.tensor_copy(out=st2[:, :W], in_=hp2[:, :W])
        stf2 = st2[:, :].bitcast(f32)
        nc.vector.tensor_copy(
            out=bass.AP(tensor=stf2.tensor, offset=stf2.offset + 32,
                        ap=[list(stf2.ap[0])] + [[1, 1]]),
            in_=hp2[:, W:W + 1])
        nc.vector.memset(st2[:, W + 2:], 0.0)
        for wi, rows in ((2 * t, slice(0, W)), (2 * t + 1, slice(W, P))):
            hi = sdw2[:, wi * 2:wi * 2 + 1]
            nc.vector.tensor_copy(out=hi, in_=hp2[rows, 65:66])
            hiF = wp.tile([W, 1], f32, tag="hiF2")
            nc.vector.tensor_copy(out=hiF[:, :], in_=hi)
            res = wp.tile([W, 1], f32, tag="res2")
            nc.vector.tensor_tensor(out=res[:, :], in0=hp2[rows, 65:66],
                                    in1=hiF[:, :], op=Alu.subtract)
            nc.vector.tensor_copy(out=sdw2[:, wi * 2 + 1:wi * 2 + 2],
                                  in_=res[:, :])
        nc.sync.dma_start(out=t2h_d[t * P:(t + 1) * P, :], in_=st2[:, :])

    nc.gpsimd.collective_compute(
        "AllGather", Alu.bypass, replica_groups=groups,
        ins=[t2h_d[:, :]], outs=[tab2_d[:, :]])

    edge_phase(2, tab2_d, 1, 65, sdw2, 65, flush2)


def kernel(**inputs):
    import concourse.bacc as bacc
    from concourse.bass_utils import run_bass_kernel_spmd

    n = inputs["x"].shape[0]
    cfg, in_maps, perm = _prep(**inputs)

    nc = bacc.Bacc("TRN2", target_bir_lowering=False, debug=False,
                   num_devices=cfg["n_cores"])
    _build(nc, cfg)
    nc.compile()

    res = run_bass_kernel_spmd(nc, in_maps,
                               core_ids=list(range(cfg["n_cores"])))
    full = np.concatenate([r["out"] for r in res.results], axis=0)
    out = full[perm[:n]]
    return np.ascontiguousarray(out, np.float32)
